# revision 16
# baseline (speedup 1.0000x reference)
"""Batched Kalman-gain kernel v2 for Trainium2 (Bass/Tile), 8-core data parallel.

Per batch b: Sigma = F Sp F^T + Q; S = H Sigma H^T + R; KG = Sigma H^T S^-1.
Factored: A = H F; C = Sp A^T; P12 = F C + (H Q)^T; S = H P12 + R;
X = S^-1 (2x2-block Schur, approx-recip); KG = P12 X.

Layout: "planes". 128 SBUF partitions = batch lanes, each lane holds G
batches per chunk. Inputs arrive batch-major [p, g, comp] (contiguous DMA);
an ACT transpose-cast pass produces fp16 component-planes [p, comp, g].
Every per-batch product term is then an elementwise TT with all operands
stride-1 innermost -> DVE 2x_1P fp16 mode (2 el/cycle/lane). Contraction
sums ride the PE via an fp16 identity stationary accumulating in PSUM
(1 col/cycle at 2.4 GHz when hot); ACT evacuates PSUM->SBUF in whatever
plane order the next stage needs. The SPD 4x4 inverse is a Schur
complement on S-planes, batched over IPAIR chunks, reciprocal_approx_fast.
"""

import os

import numpy as np

P = 128
B = 262144
NCORES = 8
B_CORE = B // NCORES  # 32768

G = int(os.environ.get("KG_G", "16"))
IPAIR = int(os.environ.get("KG_IPAIR", "8"))
ASSIGN = os.environ.get("KG_ASSIGN", "A:v,C:v,FC:v,HQ:v,S:v,KG:v")

_NC_CACHE = {}


def _build_nc(g=None, ipair=None, assign=None):
    import concourse.bacc as bacc
    import concourse.mybir as mybir
    import concourse.tile as tile
    from concourse.masks import make_identity

    g = G if g is None else g
    ipair = IPAIR if ipair is None else ipair
    assign = ASSIGN if assign is None else assign

    fp32 = mybir.dt.float32
    fp32r = mybir.dt.float32r
    fp16 = mybir.dt.float16
    MULT = mybir.AluOpType.mult

    eng_of = dict(kv.split(":") for kv in assign.split(","))

    nchunk = B_CORE // (P * g)
    assert nchunk * P * g == B_CORE
    assert nchunk % ipair == 0
    gi = g * ipair

    nc = bacc.Bacc("TRN2", target_bir_lowering=False, debug=False)

    F_d = nc.dram_tensor("F", [B_CORE, 8, 8], fp32, kind="ExternalInput").ap()
    H_d = nc.dram_tensor("H", [B_CORE, 4, 8], fp32, kind="ExternalInput").ap()
    Sp_d = nc.dram_tensor(
        "Sigma_previous", [B_CORE, 8, 8], fp32, kind="ExternalInput"
    ).ap()
    Q_d = nc.dram_tensor("Q", [B_CORE, 8, 8], fp32, kind="ExternalInput").ap()
    R_d = nc.dram_tensor("R", [B_CORE, 4, 4], fp32, kind="ExternalInput").ap()
    KG_d = nc.dram_tensor("KG", [B_CORE, 8, 4], fp32, kind="ExternalOutput").ap()

    Fv = F_d.rearrange("(c p g) i j -> c p g i j", p=P, g=g)
    Hv = H_d.rearrange("(c p g) m j -> c p g m j", p=P, g=g)
    Spv = Sp_d.rearrange("(c p g) i j -> c p g i j", p=P, g=g)
    Qv = Q_d.rearrange("(c p g) i j -> c p g i j", p=P, g=g)
    Rv = R_d.rearrange("(c p g) m n -> c p g m n", p=P, g=g)
    KGv = KG_d.rearrange("(c p g) i m -> c p g i m", p=P, g=g)

    with tile.TileContext(nc) as tc:
        with (
            tc.tile_pool(name="consts", bufs=1) as consts,
            tc.tile_pool(name="ins", bufs=2) as insp,
            tc.tile_pool(name="planes", bufs=int(os.environ.get("KG_PLB", "3"))) as plp,
            tc.tile_pool(name="prod", bufs=2) as prodp,
            tc.tile_pool(name="mid", bufs=3) as midp,
            tc.tile_pool(name="p12", bufs=IPAIR + 4) as p12p,
            tc.tile_pool(name="sx", bufs=2) as sxp,
            tc.tile_pool(name="invt", bufs=1) as invp,
            tc.tile_pool(name="out", bufs=2) as outp,
            tc.tile_pool(name="psA", bufs=int(os.environ.get("KG_PSB", "5")), space="PSUM") as psmain,
            tc.tile_pool(name="psB", bufs=2, space="PSUM") as pssml,
        ):
            ident = consts.tile([P, P], fp32, tag="ident")
            make_identity(nc, ident[:])
            id16_t = consts.tile([P, P], fp16, tag="id16")
            nc.vector.tensor_copy(id16_t[:], ident[:])
            id16 = id16_t[:]
            idr_t = consts.tile([P, P], fp32r, tag="idr")
            nc.vector.tensor_copy(idr_t[:], ident[:])
            idr = idr_t[:]

            V = nc.vector
            GP = nc.gpsimd

            def ENG(stage):
                return V if eng_of.get(stage, "v") == "v" else GP

            st = [dict() for _ in range(nchunk)]
            inv_st = [dict() for _ in range(nchunk // ipair)]

            def emit_load(c):
                s = st[c]
                s["Fn"] = insp.tile([P, g, 8, 8], fp32, tag="Fn", name="Fn")
                s["Hn"] = insp.tile([P, g, 4, 8], fp32, tag="Hn", name="Hn")
                s["Spn"] = insp.tile([P, g, 8, 8], fp32, tag="Spn", name="Spn")
                s["Qn"] = insp.tile([P, g, 8, 8], fp32, tag="Qn", name="Qn")
                s["Rn"] = insp.tile([P, g, 4, 4], fp32, tag="Rn", name="Rn")
                nc.sync.dma_start(out=s["Fn"][:], in_=Fv[c])
                nc.sync.dma_start(out=s["Hn"][:], in_=Hv[c])
                nc.sync.dma_start(out=s["Spn"][:], in_=Spv[c])
                nc.sync.dma_start(out=s["Qn"][:], in_=Qv[c])
                nc.sync.dma_start(out=s["Rn"][:], in_=Rv[c])

            def emit_transpose(c):
                s = st[c]
                s["Fp"] = plp.tile([P, 8, 8, g], fp16, tag="Fp", name="Fp")
                s["Hp"] = plp.tile([P, 4, 8, g], fp16, tag="Hp", name="Hp")
                s["Spp"] = plp.tile([P, 8, 8, g], fp16, tag="Spp", name="Spp")
                s["Qp"] = plp.tile([P, 8, 8, g], fp16, tag="Qp", name="Qp")
                s["R16"] = plp.tile([P, 4, 4, g], fp16, tag="R16", name="R16")
                nc.scalar.copy(s["Hp"][:], s["Hn"][:].rearrange("p g m j -> p m j g"))
                nc.scalar.copy(s["R16"][:], s["Rn"][:].rearrange("p g m n -> p m n g"))
                nc.scalar.copy(s["Fp"][:], s["Fn"][:].rearrange("p g i j -> p i j g"))

            def emit_transpose_tail(c):
                s = st[c]
                nc.scalar.copy(s["Spp"][:], s["Spn"][:].rearrange("p g i j -> p i j g"))
                nc.scalar.copy(s["Qp"][:], s["Qn"][:].rearrange("p g i j -> p i j g"))

            def emit_A(c):
                # A(m,cc) = sum_j Hp(m,j) Fp(j,cc); evac -> Atp planes (cc,m)
                s = st[c]
                prodA = prodp.tile([P, 4, 8, 8, g], fp16, tag="prodA", name="prodA")
                ENG("A").tensor_tensor(
                    prodA[:].rearrange("p m j cc g -> p (m j) cc g"),
                    s["Hp"][:]
                    .rearrange("p m j g -> p (m j) g")
                    .unsqueeze(2)
                    .broadcast_to([P, 32, 8, g]),
                    s["Fp"][:]
                    .rearrange("p j cc g -> p (j cc) g")
                    .unsqueeze(1)
                    .broadcast_to([P, 4, 64, g]),
                    op=MULT,
                )
                psA = psmain.tile([P, 4 * 8 * g], fp32, tag="ps", name="psA")
                for j in range(8):
                    nc.tensor.matmul(
                        psA[:],
                        id16,
                        prodA[:, :, j].rearrange("p m cc g -> p m (cc g)"),
                        start=(j == 0),
                        stop=(j == 7),
                    )
                s["Atp"] = midp.tile([P, 8, 4, g], fp16, tag="Atp", name="Atp")
                nc.scalar.copy(
                    s["Atp"][:].rearrange("p cc m g -> p m cc g"),
                    psA[:].rearrange("p (m cc g) -> p m cc g", m=4, g=g),
                )

            def emit_C(c):
                # C(i,m) = sum_cc Spp(i,cc) Atp(cc,m); evac -> Cp planes (i,m)
                s = st[c]
                prodC = prodp.tile([P, 8, 8, 4, g], fp16, tag="prodC", name="prodC")
                ENG("C").tensor_tensor(
                    prodC[:].rearrange("p i cc m g -> p (i cc) m g"),
                    s["Spp"][:]
                    .rearrange("p i cc g -> p (i cc) g")
                    .unsqueeze(2)
                    .broadcast_to([P, 64, 4, g]),
                    s["Atp"][:]
                    .rearrange("p cc m g -> p (cc m) g")
                    .unsqueeze(1)
                    .broadcast_to([P, 8, 32, g]),
                    op=MULT,
                )
                psC = psmain.tile([P, 8 * 4 * g], fp32, tag="ps", name="psC")
                for cc in range(8):
                    nc.tensor.matmul(
                        psC[:],
                        id16,
                        prodC[:, :, cc].rearrange("p i m g -> p i (m g)"),
                        start=(cc == 0),
                        stop=(cc == 7),
                    )
                s["Cp"] = midp.tile([P, 8, 4, g], fp16, tag="Cp", name="Cp")
                nc.scalar.copy(
                    s["Cp"][:].rearrange("p i m g -> p (i m) g"),
                    psC[:].rearrange("p (im g) -> p im g", g=g),
                )

            def emit_P12(c):
                # P12(i,m) = sum_n Fp(i,n) Cp(n,m) + sum_cc Hp(m,cc) Qp(cc,i)
                s = st[c]
                prodF = prodp.tile([P, 8, 8, 4, g], fp16, tag="prodF", name="prodF")
                ENG("FC").tensor_tensor(
                    prodF[:].rearrange("p i n m g -> p (i n) m g"),
                    s["Fp"][:]
                    .rearrange("p i n g -> p (i n) g")
                    .unsqueeze(2)
                    .broadcast_to([P, 64, 4, g]),
                    s["Cp"][:]
                    .rearrange("p n m g -> p (n m) g")
                    .unsqueeze(1)
                    .broadcast_to([P, 8, 32, g]),
                    op=MULT,
                )
                prodQ = prodp.tile([P, 4, 8, 8, g], fp16, tag="prodQ", name="prodQ")
                ENG("HQ").tensor_tensor(
                    prodQ[:].rearrange("p m cc i g -> p (m cc) i g"),
                    s["Hp"][:]
                    .rearrange("p m cc g -> p (m cc) g")
                    .unsqueeze(2)
                    .broadcast_to([P, 32, 8, g]),
                    s["Qp"][:]
                    .rearrange("p cc i g -> p (cc i) g")
                    .unsqueeze(1)
                    .broadcast_to([P, 4, 64, g]),
                    op=MULT,
                )
                psP = psmain.tile([P, 8 * 4 * g], fp32, tag="ps", name="psP")
                for n in range(8):
                    nc.tensor.matmul(
                        psP[:],
                        id16,
                        prodF[:, :, n].rearrange("p i m g -> p i (m g)"),
                        start=(n == 0),
                        stop=False,
                    )
                psP_mi = psP[:].rearrange("p (i m g) -> p m i g", i=8, m=4, g=g)
                for cc in range(8):
                    nc.tensor.matmul(
                        psP_mi,
                        id16,
                        prodQ[:, :, cc].rearrange("p m i g -> p m (i g)"),
                        start=False,
                        stop=(cc == 7),
                    )
                s["P12p"] = p12p.tile([P, 8, 4, g], fp16, tag="P12p", name="P12p")
                nc.scalar.copy(
                    s["P12p"][:].rearrange("p i m g -> p (i m) g"),
                    psP[:].rearrange("p (im g) -> p im g", g=g),
                )

            def emit_S(c):
                # S(m,n) = sum_i Hp(m,i) P12p(i,n) + R
                s = st[c]
                prodS = prodp.tile([P, 4, 8, 4, g], fp16, tag="prodS", name="prodS")
                ENG("S").tensor_tensor(
                    prodS[:].rearrange("p m i n g -> p (m i) n g"),
                    s["Hp"][:]
                    .rearrange("p m i g -> p (m i) g")
                    .unsqueeze(2)
                    .broadcast_to([P, 32, 4, g]),
                    s["P12p"][:]
                    .rearrange("p i n g -> p (i n) g")
                    .unsqueeze(1)
                    .broadcast_to([P, 4, 32, g]),
                    op=MULT,
                )
                psS = pssml.tile([P, 4 * 4 * g], fp32, tag="psS", name="psS")
                for i in range(8):
                    nc.tensor.matmul(
                        psS[:],
                        id16,
                        prodS[:, :, i].rearrange("p m n g -> p m (n g)"),
                        start=(i == 0),
                        stop=False,
                    )
                nc.tensor.matmul(
                    psS[:],
                    id16,
                    s["R16"][:].rearrange("p m n g -> p (m n g)"),
                    start=False,
                    stop=True,
                )
                b = c // ipair
                ph = c % ipair
                ib = inv_st[b]
                if "S2" not in ib:
                    ib["S2"] = sxp.tile([P, 16, gi], fp32, tag="S2", name="S2")
                nc.scalar.copy(
                    ib["S2"][:, :, ph * g : (ph + 1) * g],
                    psS[:].rearrange("p (q g) -> p q g", g=g),
                )

            def _qv(t):
                return t[:]

            def emit_inv(b):
                ib = inv_st[b]
                S2 = ib["S2"]
                X2 = sxp.tile([P, 16, gi], fp16, tag="X2", name="X2")
                ib["X2"] = X2
                W4 = invp.tile([P, 4, gi], fp32, tag="W4", name="W4")
                W4b = invp.tile([P, 4, gi], fp32, tag="W4b", name="W4b")
                u2 = invp.tile([P, 2, gi], fp32, tag="u2", name="u2")
                d0 = invp.tile([P, gi], fp32, tag="d0", name="d0")
                Pi = invp.tile([P, 2, 2, gi], fp32, tag="Pi", name="Pi")
                pw = invp.tile([P, 2, 2, 2, gi], fp32, tag="pw", name="pw")
                W2b = invp.tile([P, 2, 2, gi], fp32, tag="W2b", name="W2b")
                Sc = invp.tile([P, 2, 2, gi], fp32, tag="Sc", name="Sc")
                X22 = invp.tile([P, 2, 2, gi], fp32, tag="X22", name="X22")
                X21n = invp.tile([P, 2, 2, gi], fp32, tag="X21n", name="X21n")
                t4 = invp.tile([P, 2, 2, gi], fp32, tag="t4", name="t4")

                Sq = S2[:]
                Sblk = S2[:].rearrange("p (m n) g -> p m n g", m=4)

                V.tensor_tensor(u2[:], Sq[:, 0:2], Sq[:, 5:3:-1], op=MULT)
                V.tensor_sub(d0[:], u2[:, 0], u2[:, 1])
                V.reciprocal_approx_fast(out=W4[:, 0], in_=d0[:])
                V.tensor_scalar_mul(
                    W4[:, 1:3], W4[:, 0].unsqueeze(1).broadcast_to([P, 2, gi]), -1.0
                )
                V.tensor_copy(W4[:, 3], W4[:, 0])
                V.tensor_tensor(
                    Pi[:],
                    Sblk[:, 1::-1, 1::-1],
                    W4[:].rearrange("p (q r) g -> p q r g", q=2),
                    op=MULT,
                )
                Bblk = Sblk[:, 2:4, 0:2]
                for si in range(2):
                    V.tensor_tensor(
                        pw[:, si],
                        Bblk[:, :, si]
                        .unsqueeze(2)
                        .broadcast_to([P, 2, 2, gi]),
                        Pi[:, si]
                        .unsqueeze(1)
                        .broadcast_to([P, 2, 2, gi]),
                        op=MULT,
                    )
                V.tensor_add(W2b[:], pw[:, 0], pw[:, 1])
                for si in range(2):
                    V.tensor_tensor(
                        pw[:, si],
                        W2b[:, :, si]
                        .unsqueeze(2)
                        .broadcast_to([P, 2, 2, gi]),
                        Bblk[:, :, si]
                        .unsqueeze(1)
                        .broadcast_to([P, 2, 2, gi]),
                        op=MULT,
                    )
                V.tensor_add(t4[:], pw[:, 0], pw[:, 1])
                V.tensor_sub(
                    Sc[:],
                    Sblk[:, 2:4, 2:4],
                    t4[:],
                )
                Scq = Sc[:].rearrange("p q r g -> p (q r) g")
                V.tensor_tensor(u2[:], Scq[:, 0:2], Scq[:, 3:1:-1], op=MULT)
                V.tensor_sub(d0[:], u2[:, 0], u2[:, 1])
                V.reciprocal_approx_fast(out=W4b[:, 0], in_=d0[:])
                V.tensor_scalar_mul(
                    W4b[:, 1:3], W4b[:, 0].unsqueeze(1).broadcast_to([P, 2, gi]), -1.0
                )
                V.tensor_copy(W4b[:, 3], W4b[:, 0])
                ib["_cont"] = (X2, W4b, Pi, pw, W2b, Sc, X22, X21n, t4)

            def emit_inv_b(b):
                ib = inv_st[b]
                (X2, W4b, Pi, pw, W2b, Sc, X22, X21n, t4) = ib["_cont"]
                V.tensor_tensor(
                    X22[:],
                    Sc[:, 1::-1, 1::-1],
                    W4b[:].rearrange("p (q r) g -> p q r g", q=2),
                    op=MULT,
                )
                for si in range(2):
                    V.tensor_tensor(
                        pw[:, si],
                        X22[:, :, si]
                        .unsqueeze(2)
                        .broadcast_to([P, 2, 2, gi]),
                        W2b[:, si]
                        .unsqueeze(1)
                        .broadcast_to([P, 2, 2, gi]),
                        op=MULT,
                    )
                V.tensor_add(X21n[:], pw[:, 0], pw[:, 1])
                Xblk = X2[:].rearrange("p (n m) g -> p n m g", n=4)
                V.tensor_scalar_mul(Xblk[:, 2:4, 0:2], X21n[:], -1.0)
                V.tensor_scalar_mul(
                    Xblk[:, 0:2, 2:4].rearrange("p n mp g -> p mp n g"),
                    X21n[:],
                    -1.0,
                )
                V.tensor_copy(Xblk[:, 2:4, 2:4], X22[:])
                for si in range(2):
                    V.tensor_tensor(
                        pw[:, si],
                        W2b[:, si]
                        .unsqueeze(2)
                        .broadcast_to([P, 2, 2, gi]),
                        X21n[:, si]
                        .unsqueeze(1)
                        .broadcast_to([P, 2, 2, gi]),
                        op=MULT,
                    )
                V.tensor_add(t4[:], pw[:, 0], pw[:, 1])
                V.tensor_add(
                    Xblk[:, 0:2, 0:2],
                    Pi[:],
                    t4[:],
                )

            def emit_KG(c):
                s = st[c]
                b = c // ipair
                ph = c % ipair
                X2 = inv_st[b]["X2"]
                Xh = X2[:, :, ph * g : (ph + 1) * g]
                prodK = prodp.tile([P, 8, 4, 4, g], fp16, tag="prodK", name="prodK")
                ENG("KG").tensor_tensor(
                    prodK[:].rearrange("p i n m g -> p (i n) m g"),
                    s["P12p"][:]
                    .rearrange("p i n g -> p (i n) g")
                    .unsqueeze(2)
                    .broadcast_to([P, 32, 4, g]),
                    Xh.unsqueeze(1).broadcast_to([P, 8, 16, g]),
                    op=MULT,
                )
                psK = psmain.tile([P, 8 * 4 * g], fp32, tag="ps", name="psK")
                for n in range(4):
                    nc.tensor.matmul(
                        psK[:],
                        id16,
                        prodK[:, :, n].rearrange("p i m g -> p i (m g)"),
                        start=(n == 0),
                        stop=(n == 3),
                    )
                KGo = outp.tile([P, g, 8, 4], fp32, tag="KGo", name="KGo")
                nc.scalar.copy(
                    KGo[:].rearrange("p g i m -> p (i m) g"),
                    psK[:].rearrange("p (im g) -> p im g", g=g),
                )
                nc.sync.dma_start(out=KGv[c], in_=KGo[:])

            def _qv_idx(t, si):
                return t[:, si]

            # waves: L(t) | T(t-1) | A(t-2) | P12(t-3) | S(t-4) |
            #        KG(t-4-ipair) | C(t-2) | inv(group ending at t-4)
            kskew = 5 + ipair
            for t in range(nchunk + kskew + 1):
                if t < nchunk:
                    emit_load(t)
                if 0 <= t - 1 < nchunk:
                    emit_transpose(t - 1)
                if 0 <= t - 2 < nchunk:
                    emit_A(t - 2)
                if 0 <= t - 3 < nchunk:
                    emit_P12(t - 3)
                if 0 <= t - 4 < nchunk:
                    emit_S(t - 4)
                if 0 <= t - 2 < nchunk:
                    emit_C(t - 2)
                if 0 <= t - 4 < nchunk and (t - 4) % ipair == ipair - 1:
                    emit_inv((t - 4) // ipair)
                if 0 <= t - 5 < nchunk and (t - 5) % ipair == ipair - 1:
                    emit_inv_b((t - 5) // ipair)
                # two KGs per wave starting one wave after inv_b
                o = t - kskew
                if o >= 0:
                    b2 = o // ipair
                    oo = o - b2 * ipair
                    if 0 <= oo < ipair // 2:
                        for c2 in (
                            b2 * ipair + 2 * oo,
                            b2 * ipair + 2 * oo + 1,
                        ):
                            if 0 <= c2 < nchunk:
                                emit_KG(c2)
                if 0 <= t - 1 < nchunk:
                    emit_transpose_tail(t - 1)

    nc.compile()
    return nc


def _get_nc():
    if "nc" not in _NC_CACHE:
        _NC_CACHE["nc"] = _build_nc()
    return _NC_CACHE["nc"]


def kernel(F, H, Sigma_previous, Q, R):
    from concourse.bass_utils import run_bass_kernel_spmd

    nc = _get_nc()
    in_maps = []
    for ci in range(NCORES):
        sl = slice(ci * B_CORE, (ci + 1) * B_CORE)
        in_maps.append(
            {
                "F": np.ascontiguousarray(F[sl], dtype=np.float32),
                "H": np.ascontiguousarray(H[sl], dtype=np.float32),
                "Sigma_previous": np.ascontiguousarray(
                    Sigma_previous[sl], dtype=np.float32
                ),
                "Q": np.ascontiguousarray(Q[sl], dtype=np.float32),
                "R": np.ascontiguousarray(R[sl], dtype=np.float32),
            }
        )
    res = run_bass_kernel_spmd(nc, in_maps, core_ids=list(range(NCORES)))
    return np.concatenate([r["KG"] for r in res.results], axis=0)


# revision 18
# speedup vs baseline: 1.0006x; 1.0006x over previous
"""Batched Kalman-gain kernel v2 for Trainium2 (Bass/Tile), 8-core data parallel.

Per batch b: Sigma = F Sp F^T + Q; S = H Sigma H^T + R; KG = Sigma H^T S^-1.
Factored: A = H F; C = Sp A^T; P12 = F C + (H Q)^T; S = H P12 + R;
X = S^-1 (2x2-block Schur, approx-recip); KG = P12 X.

Layout: "planes". 128 SBUF partitions = batch lanes, each lane holds G
batches per chunk. Inputs arrive batch-major [p, g, comp] (contiguous DMA);
an ACT transpose-cast pass produces fp16 component-planes [p, comp, g].
Every per-batch product term is then an elementwise TT with all operands
stride-1 innermost -> DVE 2x_1P fp16 mode (2 el/cycle/lane). Contraction
sums ride the PE via an fp16 identity stationary accumulating in PSUM
(1 col/cycle at 2.4 GHz when hot); ACT evacuates PSUM->SBUF in whatever
plane order the next stage needs. The SPD 4x4 inverse is a Schur
complement on S-planes, batched over IPAIR chunks, reciprocal_approx_fast.
"""

import os

import numpy as np

P = 128
B = 262144
NCORES = 8
B_CORE = B // NCORES  # 32768

G = int(os.environ.get("KG_G", "16"))
IPAIR = int(os.environ.get("KG_IPAIR", "8"))
ASSIGN = os.environ.get("KG_ASSIGN", "A:v,C:v,FC:v,HQ:v,S:v,KG:v")

_NC_CACHE = {}


def _build_nc(g=None, ipair=None, assign=None):
    import concourse.bacc as bacc
    import concourse.mybir as mybir
    import concourse.tile as tile
    from concourse.masks import make_identity

    g = G if g is None else g
    ipair = IPAIR if ipair is None else ipair
    assign = ASSIGN if assign is None else assign

    fp32 = mybir.dt.float32
    fp32r = mybir.dt.float32r
    fp16 = mybir.dt.float16
    MULT = mybir.AluOpType.mult

    eng_of = dict(kv.split(":") for kv in assign.split(","))

    nchunk = B_CORE // (P * g)
    assert nchunk * P * g == B_CORE
    assert nchunk % ipair == 0
    gi = g * ipair

    nc = bacc.Bacc("TRN2", target_bir_lowering=False, debug=False)

    F_d = nc.dram_tensor("F", [B_CORE, 8, 8], fp32, kind="ExternalInput").ap()
    H_d = nc.dram_tensor("H", [B_CORE, 4, 8], fp32, kind="ExternalInput").ap()
    Sp_d = nc.dram_tensor(
        "Sigma_previous", [B_CORE, 8, 8], fp32, kind="ExternalInput"
    ).ap()
    Q_d = nc.dram_tensor("Q", [B_CORE, 8, 8], fp32, kind="ExternalInput").ap()
    R_d = nc.dram_tensor("R", [B_CORE, 4, 4], fp32, kind="ExternalInput").ap()
    KG_d = nc.dram_tensor("KG", [B_CORE, 8, 4], fp32, kind="ExternalOutput").ap()

    Fv = F_d.rearrange("(c p g) i j -> c p g i j", p=P, g=g)
    Hv = H_d.rearrange("(c p g) m j -> c p g m j", p=P, g=g)
    Spv = Sp_d.rearrange("(c p g) i j -> c p g i j", p=P, g=g)
    Qv = Q_d.rearrange("(c p g) i j -> c p g i j", p=P, g=g)
    Rv = R_d.rearrange("(c p g) m n -> c p g m n", p=P, g=g)
    KGv = KG_d.rearrange("(c p g) i m -> c p g i m", p=P, g=g)

    with tile.TileContext(nc) as tc:
        with (
            tc.tile_pool(name="consts", bufs=1) as consts,
            tc.tile_pool(name="ins", bufs=2) as insp,
            tc.tile_pool(name="planes", bufs=int(os.environ.get("KG_PLB", "3"))) as plp,
            tc.tile_pool(name="prod", bufs=2) as prodp,
            tc.tile_pool(name="mid", bufs=3) as midp,
            tc.tile_pool(name="p12", bufs=IPAIR + 4) as p12p,
            tc.tile_pool(name="sx", bufs=2) as sxp,
            tc.tile_pool(name="invt", bufs=1) as invp,
            tc.tile_pool(name="out", bufs=2) as outp,
            tc.tile_pool(name="psA", bufs=int(os.environ.get("KG_PSB", "5")), space="PSUM") as psmain,
            tc.tile_pool(name="psB", bufs=2, space="PSUM") as pssml,
        ):
            ident = consts.tile([P, P], fp32, tag="ident")
            make_identity(nc, ident[:])
            id16_t = consts.tile([P, P], fp16, tag="id16")
            nc.vector.tensor_copy(id16_t[:], ident[:])
            id16 = id16_t[:]
            idr_t = consts.tile([P, P], fp32r, tag="idr")
            nc.vector.tensor_copy(idr_t[:], ident[:])
            idr = idr_t[:]

            V = nc.vector
            GP = nc.gpsimd

            def ENG(stage):
                return V if eng_of.get(stage, "v") == "v" else GP

            st = [dict() for _ in range(nchunk)]
            inv_st = [dict() for _ in range(nchunk // ipair)]

            def emit_load(c):
                s = st[c]
                s["Fn"] = insp.tile([P, g, 8, 8], fp32, tag="Fn", name="Fn")
                s["Hn"] = insp.tile([P, g, 4, 8], fp32, tag="Hn", name="Hn")
                s["Spn"] = insp.tile([P, g, 8, 8], fp32, tag="Spn", name="Spn")
                s["Qn"] = insp.tile([P, g, 8, 8], fp32, tag="Qn", name="Qn")
                s["Rn"] = insp.tile([P, g, 4, 4], fp32, tag="Rn", name="Rn")
                nc.sync.dma_start(out=s["Fn"][:], in_=Fv[c])
                nc.sync.dma_start(out=s["Hn"][:], in_=Hv[c])
                nc.sync.dma_start(out=s["Spn"][:], in_=Spv[c])
                nc.sync.dma_start(out=s["Qn"][:], in_=Qv[c])
                nc.sync.dma_start(out=s["Rn"][:], in_=Rv[c])

            def emit_transpose(c):
                s = st[c]
                s["Fp"] = plp.tile([P, 8, 8, g], fp16, tag="Fp", name="Fp")
                s["Hp"] = plp.tile([P, 4, 8, g], fp16, tag="Hp", name="Hp")
                s["Spp"] = plp.tile([P, 8, 8, g], fp16, tag="Spp", name="Spp")
                s["Qp"] = plp.tile([P, 8, 8, g], fp16, tag="Qp", name="Qp")
                s["R16"] = plp.tile([P, 4, 4, g], fp16, tag="R16", name="R16")
                nc.scalar.copy(s["Hp"][:], s["Hn"][:].rearrange("p g m j -> p m j g"))
                nc.scalar.copy(s["R16"][:], s["Rn"][:].rearrange("p g m n -> p m n g"))
                nc.scalar.copy(s["Fp"][:], s["Fn"][:].rearrange("p g i j -> p i j g"))

            def emit_transpose_tail(c):
                s = st[c]
                nc.scalar.copy(s["Spp"][:], s["Spn"][:].rearrange("p g i j -> p i j g"))
                nc.scalar.copy(s["Qp"][:], s["Qn"][:].rearrange("p g i j -> p i j g"))

            def emit_A(c):
                # A(m,cc) = sum_j Hp(m,j) Fp(j,cc); evac -> Atp planes (cc,m)
                s = st[c]
                prodA = prodp.tile([P, 4, 8, 8, g], fp16, tag="prodA", name="prodA")
                ENG("A").tensor_tensor(
                    prodA[:].rearrange("p m j cc g -> p (m j) cc g"),
                    s["Hp"][:]
                    .rearrange("p m j g -> p (m j) g")
                    .unsqueeze(2)
                    .broadcast_to([P, 32, 8, g]),
                    s["Fp"][:]
                    .rearrange("p j cc g -> p (j cc) g")
                    .unsqueeze(1)
                    .broadcast_to([P, 4, 64, g]),
                    op=MULT,
                )
                psA = psmain.tile([P, 4 * 8 * g], fp32, tag="ps", name="psA")
                for j in range(8):
                    nc.tensor.matmul(
                        psA[:],
                        id16,
                        prodA[:, :, j].rearrange("p m cc g -> p m (cc g)"),
                        start=(j == 0),
                        stop=(j == 7),
                    )
                s["Atp"] = midp.tile([P, 8, 4, g], fp16, tag="Atp", name="Atp")
                nc.scalar.copy(
                    s["Atp"][:].rearrange("p cc m g -> p m cc g"),
                    psA[:].rearrange("p (m cc g) -> p m cc g", m=4, g=g),
                )

            def emit_C(c):
                # C(i,m) = sum_cc Spp(i,cc) Atp(cc,m); evac -> Cp planes (i,m)
                s = st[c]
                prodC = prodp.tile([P, 8, 8, 4, g], fp16, tag="prodC", name="prodC")
                ENG("C").tensor_tensor(
                    prodC[:].rearrange("p i cc m g -> p (i cc) m g"),
                    s["Spp"][:]
                    .rearrange("p i cc g -> p (i cc) g")
                    .unsqueeze(2)
                    .broadcast_to([P, 64, 4, g]),
                    s["Atp"][:]
                    .rearrange("p cc m g -> p (cc m) g")
                    .unsqueeze(1)
                    .broadcast_to([P, 8, 32, g]),
                    op=MULT,
                )
                psC = psmain.tile([P, 8 * 4 * g], fp32, tag="ps", name="psC")
                for cc in range(8):
                    nc.tensor.matmul(
                        psC[:],
                        id16,
                        prodC[:, :, cc].rearrange("p i m g -> p i (m g)"),
                        start=(cc == 0),
                        stop=(cc == 7),
                    )
                s["Cp"] = midp.tile([P, 8, 4, g], fp16, tag="Cp", name="Cp")
                nc.scalar.copy(
                    s["Cp"][:].rearrange("p i m g -> p (i m) g"),
                    psC[:].rearrange("p (im g) -> p im g", g=g),
                )

            def emit_P12(c):
                # P12(i,m) = sum_n Fp(i,n) Cp(n,m) + sum_cc Hp(m,cc) Qp(cc,i)
                s = st[c]
                prodF = prodp.tile([P, 8, 8, 4, g], fp16, tag="prodF", name="prodF")
                ENG("FC").tensor_tensor(
                    prodF[:].rearrange("p i n m g -> p (i n) m g"),
                    s["Fp"][:]
                    .rearrange("p i n g -> p (i n) g")
                    .unsqueeze(2)
                    .broadcast_to([P, 64, 4, g]),
                    s["Cp"][:]
                    .rearrange("p n m g -> p (n m) g")
                    .unsqueeze(1)
                    .broadcast_to([P, 8, 32, g]),
                    op=MULT,
                )
                prodQ = prodp.tile([P, 4, 8, 8, g], fp16, tag="prodQ", name="prodQ")
                ENG("HQ").tensor_tensor(
                    prodQ[:].rearrange("p m cc i g -> p (m cc) i g"),
                    s["Hp"][:]
                    .rearrange("p m cc g -> p (m cc) g")
                    .unsqueeze(2)
                    .broadcast_to([P, 32, 8, g]),
                    s["Qp"][:]
                    .rearrange("p cc i g -> p (cc i) g")
                    .unsqueeze(1)
                    .broadcast_to([P, 4, 64, g]),
                    op=MULT,
                )
                psP = psmain.tile([P, 8 * 4 * g], fp32, tag="ps", name="psP")
                for n in range(8):
                    nc.tensor.matmul(
                        psP[:],
                        id16,
                        prodF[:, :, n].rearrange("p i m g -> p i (m g)"),
                        start=(n == 0),
                        stop=False,
                    )
                psP_mi = psP[:].rearrange("p (i m g) -> p m i g", i=8, m=4, g=g)
                for cc in range(8):
                    nc.tensor.matmul(
                        psP_mi,
                        id16,
                        prodQ[:, :, cc].rearrange("p m i g -> p m (i g)"),
                        start=False,
                        stop=(cc == 7),
                    )
                s["P12p"] = p12p.tile([P, 8, 4, g], fp16, tag="P12p", name="P12p")
                nc.scalar.copy(
                    s["P12p"][:].rearrange("p i m g -> p (i m) g"),
                    psP[:].rearrange("p (im g) -> p im g", g=g),
                )

            def emit_S(c):
                # S(m,n) = sum_i Hp(m,i) P12p(i,n) + R
                s = st[c]
                prodS = prodp.tile([P, 4, 8, 4, g], fp16, tag="prodS", name="prodS")
                ENG("S").tensor_tensor(
                    prodS[:].rearrange("p m i n g -> p (m i) n g"),
                    s["Hp"][:]
                    .rearrange("p m i g -> p (m i) g")
                    .unsqueeze(2)
                    .broadcast_to([P, 32, 4, g]),
                    s["P12p"][:]
                    .rearrange("p i n g -> p (i n) g")
                    .unsqueeze(1)
                    .broadcast_to([P, 4, 32, g]),
                    op=MULT,
                )
                psS = pssml.tile([P, 4 * 4 * g], fp32, tag="psS", name="psS")
                for i in range(8):
                    nc.tensor.matmul(
                        psS[:],
                        id16,
                        prodS[:, :, i].rearrange("p m n g -> p m (n g)"),
                        start=(i == 0),
                        stop=False,
                    )
                nc.tensor.matmul(
                    psS[:],
                    id16,
                    s["R16"][:].rearrange("p m n g -> p (m n g)"),
                    start=False,
                    stop=True,
                )
                b = c // ipair
                ph = c % ipair
                ib = inv_st[b]
                if "S2" not in ib:
                    ib["S2"] = sxp.tile([P, 16, gi], fp32, tag="S2", name="S2")
                nc.scalar.copy(
                    ib["S2"][:, :, ph * g : (ph + 1) * g],
                    psS[:].rearrange("p (q g) -> p q g", g=g),
                )

            def _qv(t):
                return t[:]

            def emit_inv(b):
                ib = inv_st[b]
                S2 = ib["S2"]
                X2 = sxp.tile([P, 16, gi], fp16, tag="X2", name="X2")
                ib["X2"] = X2
                W4 = invp.tile([P, 4, gi], fp32, tag="W4", name="W4")
                W4b = invp.tile([P, 4, gi], fp32, tag="W4b", name="W4b")
                u2 = invp.tile([P, 2, gi], fp32, tag="u2", name="u2")
                d0 = invp.tile([P, gi], fp32, tag="d0", name="d0")
                Pi = invp.tile([P, 2, 2, gi], fp32, tag="Pi", name="Pi")
                pw = invp.tile([P, 2, 2, 2, gi], fp32, tag="pw", name="pw")
                W2b = invp.tile([P, 2, 2, gi], fp32, tag="W2b", name="W2b")
                Sc = invp.tile([P, 2, 2, gi], fp32, tag="Sc", name="Sc")
                X22 = invp.tile([P, 2, 2, gi], fp32, tag="X22", name="X22")
                X21n = invp.tile([P, 2, 2, gi], fp32, tag="X21n", name="X21n")
                t4 = invp.tile([P, 2, 2, gi], fp32, tag="t4", name="t4")

                Sq = S2[:]
                Sblk = S2[:].rearrange("p (m n) g -> p m n g", m=4)

                V.tensor_tensor(u2[:], Sq[:, 0:2], Sq[:, 5:3:-1], op=MULT)
                V.tensor_sub(d0[:], u2[:, 0], u2[:, 1])
                V.reciprocal_approx_fast(out=W4[:, 0], in_=d0[:])
                V.tensor_scalar_mul(
                    W4[:, 1:3], W4[:, 0].unsqueeze(1).broadcast_to([P, 2, gi]), -1.0
                )
                V.tensor_copy(W4[:, 3], W4[:, 0])
                V.tensor_tensor(
                    Pi[:],
                    Sblk[:, 1::-1, 1::-1],
                    W4[:].rearrange("p (q r) g -> p q r g", q=2),
                    op=MULT,
                )
                Bblk = Sblk[:, 2:4, 0:2]
                for si in range(2):
                    V.tensor_tensor(
                        pw[:, si],
                        Bblk[:, :, si]
                        .unsqueeze(2)
                        .broadcast_to([P, 2, 2, gi]),
                        Pi[:, si]
                        .unsqueeze(1)
                        .broadcast_to([P, 2, 2, gi]),
                        op=MULT,
                    )
                V.tensor_add(W2b[:], pw[:, 0], pw[:, 1])
                for si in range(2):
                    V.tensor_tensor(
                        pw[:, si],
                        W2b[:, :, si]
                        .unsqueeze(2)
                        .broadcast_to([P, 2, 2, gi]),
                        Bblk[:, :, si]
                        .unsqueeze(1)
                        .broadcast_to([P, 2, 2, gi]),
                        op=MULT,
                    )
                V.tensor_add(t4[:], pw[:, 0], pw[:, 1])
                V.tensor_sub(
                    Sc[:],
                    Sblk[:, 2:4, 2:4],
                    t4[:],
                )
                Scq = Sc[:].rearrange("p q r g -> p (q r) g")
                V.tensor_tensor(u2[:], Scq[:, 0:2], Scq[:, 3:1:-1], op=MULT)
                V.tensor_sub(d0[:], u2[:, 0], u2[:, 1])
                V.reciprocal_approx_fast(out=W4b[:, 0], in_=d0[:])
                V.tensor_scalar_mul(
                    W4b[:, 1:3], W4b[:, 0].unsqueeze(1).broadcast_to([P, 2, gi]), -1.0
                )
                V.tensor_copy(W4b[:, 3], W4b[:, 0])
                ib["_cont"] = (X2, W4b, Pi, pw, W2b, Sc, X22, X21n, t4)

            def emit_inv_b(b):
                ib = inv_st[b]
                (X2, W4b, Pi, pw, W2b, Sc, X22, X21n, t4) = ib["_cont"]
                V.tensor_tensor(
                    X22[:],
                    Sc[:, 1::-1, 1::-1],
                    W4b[:].rearrange("p (q r) g -> p q r g", q=2),
                    op=MULT,
                )
                for si in range(2):
                    V.tensor_tensor(
                        pw[:, si],
                        X22[:, :, si]
                        .unsqueeze(2)
                        .broadcast_to([P, 2, 2, gi]),
                        W2b[:, si]
                        .unsqueeze(1)
                        .broadcast_to([P, 2, 2, gi]),
                        op=MULT,
                    )
                V.tensor_add(X21n[:], pw[:, 0], pw[:, 1])
                Xblk = X2[:].rearrange("p (n m) g -> p n m g", n=4)
                V.tensor_scalar_mul(Xblk[:, 2:4, 0:2], X21n[:], -1.0)
                V.tensor_scalar_mul(
                    Xblk[:, 0:2, 2:4].rearrange("p n mp g -> p mp n g"),
                    X21n[:],
                    -1.0,
                )
                V.tensor_copy(Xblk[:, 2:4, 2:4], X22[:])
                for si in range(2):
                    V.tensor_tensor(
                        pw[:, si],
                        W2b[:, si]
                        .unsqueeze(2)
                        .broadcast_to([P, 2, 2, gi]),
                        X21n[:, si]
                        .unsqueeze(1)
                        .broadcast_to([P, 2, 2, gi]),
                        op=MULT,
                    )
                V.tensor_add(t4[:], pw[:, 0], pw[:, 1])
                V.tensor_add(
                    Xblk[:, 0:2, 0:2],
                    Pi[:],
                    t4[:],
                )

            def emit_KG(c):
                s = st[c]
                b = c // ipair
                ph = c % ipair
                X2 = inv_st[b]["X2"]
                Xh = X2[:, :, ph * g : (ph + 1) * g]
                prodK = prodp.tile([P, 8, 4, 4, g], fp16, tag="prodK", name="prodK")
                ENG("KG").tensor_tensor(
                    prodK[:].rearrange("p i n m g -> p (i n) m g"),
                    s["P12p"][:]
                    .rearrange("p i n g -> p (i n) g")
                    .unsqueeze(2)
                    .broadcast_to([P, 32, 4, g]),
                    Xh.unsqueeze(1).broadcast_to([P, 8, 16, g]),
                    op=MULT,
                )
                psK = psmain.tile([P, 8 * 4 * g], fp32, tag="ps", name="psK")
                for n in range(4):
                    nc.tensor.matmul(
                        psK[:],
                        id16,
                        prodK[:, :, n].rearrange("p i m g -> p i (m g)"),
                        start=(n == 0),
                        stop=(n == 3),
                    )
                KGo = outp.tile([P, g, 8, 4], fp32, tag="KGo", name="KGo")
                nc.scalar.copy(
                    KGo[:].rearrange("p g i m -> p (i m) g"),
                    psK[:].rearrange("p (im g) -> p im g", g=g),
                )
                nc.sync.dma_start(out=KGv[c], in_=KGo[:])

            def _qv_idx(t, si):
                return t[:, si]

            # waves: L(t) | T(t-1) | A(t-2) | P12(t-3) | S(t-4) |
            #        KG(t-4-ipair) | C(t-2) | inv(group ending at t-4)
            kskew = 5 + ipair
            for t in range(nchunk + kskew + 1):
                if t < nchunk:
                    emit_load(t)
                if 0 <= t - 1 < nchunk:
                    emit_transpose(t - 1)
                if 0 <= t - 2 < nchunk:
                    emit_A(t - 2)
                if 0 <= t - 3 < nchunk:
                    emit_P12(t - 3)
                if 0 <= t - 4 < nchunk:
                    emit_S(t - 4)
                if 0 <= t - 2 < nchunk:
                    emit_C(t - 2)
                if 0 <= t - 4 < nchunk and (t - 4) % ipair == ipair - 1:
                    emit_inv((t - 4) // ipair)
                if 0 <= t - 5 < nchunk and (t - 5) % ipair == ipair - 1:
                    emit_inv_b((t - 5) // ipair)
                # two KGs per wave starting one wave after inv_b
                o = t - kskew
                if o >= 0:
                    b2 = o // ipair
                    oo = o - b2 * ipair
                    if 0 <= oo < ipair // 2:
                        for c2 in (
                            b2 * ipair + 2 * oo,
                            b2 * ipair + 2 * oo + 1,
                        ):
                            if 0 <= c2 < nchunk:
                                emit_KG(c2)
                if 0 <= t - 1 < nchunk:
                    emit_transpose_tail(t - 1)

    nc.compile()
    return nc


def _get_nc():
    if "nc" not in _NC_CACHE:
        _NC_CACHE["nc"] = _build_nc()
    return _NC_CACHE["nc"]


def kernel(F, H, Sigma_previous, Q, R):
    from concourse.bass_utils import run_bass_kernel_spmd

    nc = _get_nc()
    in_maps = []
    for ci in range(NCORES):
        sl = slice(ci * B_CORE, (ci + 1) * B_CORE)
        in_maps.append(
            {
                "F": np.ascontiguousarray(F[sl], dtype=np.float32),
                "H": np.ascontiguousarray(H[sl], dtype=np.float32),
                "Sigma_previous": np.ascontiguousarray(
                    Sigma_previous[sl], dtype=np.float32
                ),
                "Q": np.ascontiguousarray(Q[sl], dtype=np.float32),
                "R": np.ascontiguousarray(R[sl], dtype=np.float32),
            }
        )
    res = run_bass_kernel_spmd(nc, in_maps, core_ids=list(range(NCORES)))
    return np.concatenate([r["KG"] for r in res.results], axis=0)


# revision 19
# speedup vs baseline: 1.0026x; 1.0020x over previous
"""Batched Kalman-gain kernel v2 for Trainium2 (Bass/Tile), 8-core data parallel.

Per batch b: Sigma = F Sp F^T + Q; S = H Sigma H^T + R; KG = Sigma H^T S^-1.
Factored: A = H F; C = Sp A^T; P12 = F C + (H Q)^T; S = H P12 + R;
X = S^-1 (2x2-block Schur, approx-recip); KG = P12 X.

Layout: "planes". 128 SBUF partitions = batch lanes, each lane holds G
batches per chunk. Inputs arrive batch-major [p, g, comp] (contiguous DMA);
an ACT transpose-cast pass produces fp16 component-planes [p, comp, g].
Every per-batch product term is then an elementwise TT with all operands
stride-1 innermost -> DVE 2x_1P fp16 mode (2 el/cycle/lane). Contraction
sums ride the PE via an fp16 identity stationary accumulating in PSUM
(1 col/cycle at 2.4 GHz when hot); ACT evacuates PSUM->SBUF in whatever
plane order the next stage needs. The SPD 4x4 inverse is a Schur
complement on S-planes, batched over IPAIR chunks, reciprocal_approx_fast.
"""

import os

import numpy as np

P = 128
B = 262144
NCORES = 8
B_CORE = B // NCORES  # 32768

G = int(os.environ.get("KG_G", "16"))
IPAIR = int(os.environ.get("KG_IPAIR", "8"))
ASSIGN = os.environ.get("KG_ASSIGN", "A:v,C:v,FC:v,HQ:v,S:v,KG:v")

_NC_CACHE = {}


def _build_nc(g=None, ipair=None, assign=None):
    import concourse.bacc as bacc
    import concourse.mybir as mybir
    import concourse.tile as tile
    from concourse.masks import make_identity

    g = G if g is None else g
    ipair = IPAIR if ipair is None else ipair
    assign = ASSIGN if assign is None else assign

    fp32 = mybir.dt.float32
    fp32r = mybir.dt.float32r
    fp16 = mybir.dt.float16
    MULT = mybir.AluOpType.mult

    eng_of = dict(kv.split(":") for kv in assign.split(","))

    nchunk = B_CORE // (P * g)
    assert nchunk * P * g == B_CORE
    assert nchunk % ipair == 0
    gi = g * ipair

    nc = bacc.Bacc("TRN2", target_bir_lowering=False, debug=False)

    F_d = nc.dram_tensor("F", [B_CORE, 8, 8], fp32, kind="ExternalInput").ap()
    H_d = nc.dram_tensor("H", [B_CORE, 4, 8], fp32, kind="ExternalInput").ap()
    Sp_d = nc.dram_tensor(
        "Sigma_previous", [B_CORE, 8, 8], fp32, kind="ExternalInput"
    ).ap()
    Q_d = nc.dram_tensor("Q", [B_CORE, 8, 8], fp32, kind="ExternalInput").ap()
    R_d = nc.dram_tensor("R", [B_CORE, 4, 4], fp32, kind="ExternalInput").ap()
    KG_d = nc.dram_tensor("KG", [B_CORE, 8, 4], fp32, kind="ExternalOutput").ap()

    Fv = F_d.rearrange("(c p g) i j -> c p g i j", p=P, g=g)
    Hv = H_d.rearrange("(c p g) m j -> c p g m j", p=P, g=g)
    Spv = Sp_d.rearrange("(c p g) i j -> c p g i j", p=P, g=g)
    Qv = Q_d.rearrange("(c p g) i j -> c p g i j", p=P, g=g)
    Rv = R_d.rearrange("(c p g) m n -> c p g m n", p=P, g=g)
    KGv = KG_d.rearrange("(c p g) i m -> c p g i m", p=P, g=g)

    with tile.TileContext(nc) as tc:
        with (
            tc.tile_pool(name="consts", bufs=1) as consts,
            tc.tile_pool(name="ins", bufs=2) as insp,
            tc.tile_pool(name="planes", bufs=int(os.environ.get("KG_PLB", "3"))) as plp,
            tc.tile_pool(name="prod", bufs=2) as prodp,
            tc.tile_pool(name="mid", bufs=3) as midp,
            tc.tile_pool(name="p12", bufs=IPAIR + 4) as p12p,
            tc.tile_pool(name="sx", bufs=2) as sxp,
            tc.tile_pool(name="invt", bufs=1) as invp,
            tc.tile_pool(name="out", bufs=2) as outp,
            tc.tile_pool(name="psA", bufs=int(os.environ.get("KG_PSB", "5")), space="PSUM") as psmain,
            tc.tile_pool(name="psB", bufs=2, space="PSUM") as pssml,
        ):
            ident = consts.tile([P, P], fp32, tag="ident")
            make_identity(nc, ident[:])
            id16_t = consts.tile([P, P], fp16, tag="id16")
            nc.vector.tensor_copy(id16_t[:], ident[:])
            id16 = id16_t[:]
            idr_t = consts.tile([P, P], fp32r, tag="idr")
            nc.vector.tensor_copy(idr_t[:], ident[:])
            idr = idr_t[:]

            V = nc.vector
            GP = nc.gpsimd

            def ENG(stage):
                return V if eng_of.get(stage, "v") == "v" else GP

            st = [dict() for _ in range(nchunk)]
            inv_st = [dict() for _ in range(nchunk // ipair)]

            def emit_load(c):
                s = st[c]
                s["Fn"] = insp.tile([P, g, 8, 8], fp32, tag="Fn", name="Fn")
                s["Hn"] = insp.tile([P, g, 4, 8], fp32, tag="Hn", name="Hn")
                s["Spn"] = insp.tile([P, g, 8, 8], fp32, tag="Spn", name="Spn")
                s["Qn"] = insp.tile([P, g, 8, 8], fp32, tag="Qn", name="Qn")
                s["Rn"] = insp.tile([P, g, 4, 4], fp32, tag="Rn", name="Rn")
                nc.sync.dma_start(out=s["Fn"][:], in_=Fv[c])
                nc.sync.dma_start(out=s["Hn"][:], in_=Hv[c])
                nc.sync.dma_start(out=s["Spn"][:], in_=Spv[c])
                nc.sync.dma_start(out=s["Qn"][:], in_=Qv[c])
                nc.sync.dma_start(out=s["Rn"][:], in_=Rv[c])

            def emit_transpose(c):
                s = st[c]
                s["Fp"] = plp.tile([P, 8, 8, g], fp16, tag="Fp", name="Fp")
                s["Hp"] = plp.tile([P, 4, 8, g], fp16, tag="Hp", name="Hp")
                s["Spp"] = plp.tile([P, 8, 8, g], fp16, tag="Spp", name="Spp")
                s["Qp"] = plp.tile([P, 8, 8, g], fp16, tag="Qp", name="Qp")
                s["R16"] = plp.tile([P, 4, 4, g], fp16, tag="R16", name="R16")
                nc.scalar.copy(s["Hp"][:], s["Hn"][:].rearrange("p g m j -> p m j g"))
                nc.scalar.copy(s["R16"][:], s["Rn"][:].rearrange("p g m n -> p m n g"))
                nc.scalar.copy(s["Fp"][:], s["Fn"][:].rearrange("p g i j -> p i j g"))

            def emit_transpose_tail(c):
                s = st[c]
                nc.scalar.copy(s["Spp"][:], s["Spn"][:].rearrange("p g i j -> p i j g"))
                nc.scalar.copy(s["Qp"][:], s["Qn"][:].rearrange("p g i j -> p i j g"))

            def emit_A(c):
                # A(m,cc) = sum_j Hp(m,j) Fp(j,cc); evac -> Atp planes (cc,m)
                s = st[c]
                prodA = prodp.tile([P, 4, 8, 8, g], fp16, tag="prodA", name="prodA")
                ENG("A").tensor_tensor(
                    prodA[:].rearrange("p m j cc g -> p (m j) cc g"),
                    s["Hp"][:]
                    .rearrange("p m j g -> p (m j) g")
                    .unsqueeze(2)
                    .broadcast_to([P, 32, 8, g]),
                    s["Fp"][:]
                    .rearrange("p j cc g -> p (j cc) g")
                    .unsqueeze(1)
                    .broadcast_to([P, 4, 64, g]),
                    op=MULT,
                )
                psA = psmain.tile([P, 4 * 8 * g], fp32, tag="ps", name="psA")
                for j in range(8):
                    nc.tensor.matmul(
                        psA[:],
                        id16,
                        prodA[:, :, j].rearrange("p m cc g -> p m (cc g)"),
                        start=(j == 0),
                        stop=(j == 7),
                    )
                s["Atp"] = midp.tile([P, 8, 4, g], fp16, tag="Atp", name="Atp")
                nc.scalar.copy(
                    s["Atp"][:].rearrange("p cc m g -> p m cc g"),
                    psA[:].rearrange("p (m cc g) -> p m cc g", m=4, g=g),
                )

            def emit_C(c):
                # C(i,m) = sum_cc Spp(i,cc) Atp(cc,m); evac -> Cp planes (i,m)
                s = st[c]
                prodC = prodp.tile([P, 8, 8, 4, g], fp16, tag="prodC", name="prodC")
                ENG("C").tensor_tensor(
                    prodC[:].rearrange("p i cc m g -> p (i cc) m g"),
                    s["Spp"][:]
                    .rearrange("p i cc g -> p (i cc) g")
                    .unsqueeze(2)
                    .broadcast_to([P, 64, 4, g]),
                    s["Atp"][:]
                    .rearrange("p cc m g -> p (cc m) g")
                    .unsqueeze(1)
                    .broadcast_to([P, 8, 32, g]),
                    op=MULT,
                )
                psC = psmain.tile([P, 8 * 4 * g], fp32, tag="ps", name="psC")
                for cc in range(8):
                    nc.tensor.matmul(
                        psC[:],
                        id16,
                        prodC[:, :, cc].rearrange("p i m g -> p i (m g)"),
                        start=(cc == 0),
                        stop=(cc == 7),
                    )
                s["Cp"] = midp.tile([P, 8, 4, g], fp16, tag="Cp", name="Cp")
                nc.scalar.copy(
                    s["Cp"][:].rearrange("p i m g -> p (i m) g"),
                    psC[:].rearrange("p (im g) -> p im g", g=g),
                )

            def emit_P12(c):
                # P12(i,m) = sum_n Fp(i,n) Cp(n,m) + sum_cc Hp(m,cc) Qp(cc,i)
                s = st[c]
                prodF = prodp.tile([P, 8, 8, 4, g], fp16, tag="prodF", name="prodF")
                ENG("FC").tensor_tensor(
                    prodF[:].rearrange("p i n m g -> p (i n) m g"),
                    s["Fp"][:]
                    .rearrange("p i n g -> p (i n) g")
                    .unsqueeze(2)
                    .broadcast_to([P, 64, 4, g]),
                    s["Cp"][:]
                    .rearrange("p n m g -> p (n m) g")
                    .unsqueeze(1)
                    .broadcast_to([P, 8, 32, g]),
                    op=MULT,
                )
                prodQ = prodp.tile([P, 4, 8, 8, g], fp16, tag="prodQ", name="prodQ")
                ENG("HQ").tensor_tensor(
                    prodQ[:].rearrange("p m cc i g -> p (m cc) i g"),
                    s["Hp"][:]
                    .rearrange("p m cc g -> p (m cc) g")
                    .unsqueeze(2)
                    .broadcast_to([P, 32, 8, g]),
                    s["Qp"][:]
                    .rearrange("p cc i g -> p (cc i) g")
                    .unsqueeze(1)
                    .broadcast_to([P, 4, 64, g]),
                    op=MULT,
                )
                psP = psmain.tile([P, 8 * 4 * g], fp32, tag="ps", name="psP")
                for n in range(8):
                    nc.tensor.matmul(
                        psP[:],
                        id16,
                        prodF[:, :, n].rearrange("p i m g -> p i (m g)"),
                        start=(n == 0),
                        stop=False,
                    )
                psP_mi = psP[:].rearrange("p (i m g) -> p m i g", i=8, m=4, g=g)
                for cc in range(8):
                    nc.tensor.matmul(
                        psP_mi,
                        id16,
                        prodQ[:, :, cc].rearrange("p m i g -> p m (i g)"),
                        start=False,
                        stop=(cc == 7),
                    )
                s["P12p"] = p12p.tile([P, 8, 4, g], fp16, tag="P12p", name="P12p")
                nc.scalar.copy(
                    s["P12p"][:].rearrange("p i m g -> p (i m) g"),
                    psP[:].rearrange("p (im g) -> p im g", g=g),
                )

            def emit_S(c):
                # S(m,n) = sum_i Hp(m,i) P12p(i,n) + R
                s = st[c]
                prodS = prodp.tile([P, 4, 8, 4, g], fp16, tag="prodS", name="prodS")
                ENG("S").tensor_tensor(
                    prodS[:].rearrange("p m i n g -> p (m i) n g"),
                    s["Hp"][:]
                    .rearrange("p m i g -> p (m i) g")
                    .unsqueeze(2)
                    .broadcast_to([P, 32, 4, g]),
                    s["P12p"][:]
                    .rearrange("p i n g -> p (i n) g")
                    .unsqueeze(1)
                    .broadcast_to([P, 4, 32, g]),
                    op=MULT,
                )
                psS = pssml.tile([P, 4 * 4 * g], fp32, tag="psS", name="psS")
                for i in range(8):
                    nc.tensor.matmul(
                        psS[:],
                        id16,
                        prodS[:, :, i].rearrange("p m n g -> p m (n g)"),
                        start=(i == 0),
                        stop=False,
                    )
                nc.tensor.matmul(
                    psS[:],
                    id16,
                    s["R16"][:].rearrange("p m n g -> p (m n g)"),
                    start=False,
                    stop=True,
                )
                b = c // ipair
                ph = c % ipair
                ib = inv_st[b]
                if "S2" not in ib:
                    ib["S2"] = sxp.tile([P, 16, gi], fp32, tag="S2", name="S2")
                nc.scalar.copy(
                    ib["S2"][:, :, ph * g : (ph + 1) * g],
                    psS[:].rearrange("p (q g) -> p q g", g=g),
                )

            def _qv(t):
                return t[:]

            def emit_inv(b):
                ib = inv_st[b]
                S2 = ib["S2"]
                X2 = sxp.tile([P, 16, gi], fp16, tag="X2", name="X2")
                ib["X2"] = X2
                W4 = invp.tile([P, 4, gi], fp32, tag="W4", name="W4")
                W4b = invp.tile([P, 4, gi], fp32, tag="W4b", name="W4b")
                u2 = invp.tile([P, 2, gi], fp32, tag="u2", name="u2")
                d0 = invp.tile([P, gi], fp32, tag="d0", name="d0")
                Pi = invp.tile([P, 2, 2, gi], fp32, tag="Pi", name="Pi")
                pw = invp.tile([P, 2, 2, 2, gi], fp32, tag="pw", name="pw")
                W2b = invp.tile([P, 2, 2, gi], fp32, tag="W2b", name="W2b")
                Sc = invp.tile([P, 2, 2, gi], fp32, tag="Sc", name="Sc")
                X22 = invp.tile([P, 2, 2, gi], fp32, tag="X22", name="X22")
                X21n = invp.tile([P, 2, 2, gi], fp32, tag="X21n", name="X21n")
                t4 = invp.tile([P, 2, 2, gi], fp32, tag="t4", name="t4")

                Sq = S2[:]
                Sblk = S2[:].rearrange("p (m n) g -> p m n g", m=4)

                V.tensor_tensor(u2[:], Sq[:, 0:2], Sq[:, 5:3:-1], op=MULT)
                V.tensor_sub(d0[:], u2[:, 0], u2[:, 1])
                V.reciprocal_approx_fast(out=W4[:, 0], in_=d0[:])
                V.tensor_scalar_mul(
                    W4[:, 1:3], W4[:, 0].unsqueeze(1).broadcast_to([P, 2, gi]), -1.0
                )
                V.tensor_copy(W4[:, 3], W4[:, 0])
                V.tensor_tensor(
                    Pi[:],
                    Sblk[:, 1::-1, 1::-1],
                    W4[:].rearrange("p (q r) g -> p q r g", q=2),
                    op=MULT,
                )
                Bblk = Sblk[:, 2:4, 0:2]
                for si in range(2):
                    V.tensor_tensor(
                        pw[:, si],
                        Bblk[:, :, si]
                        .unsqueeze(2)
                        .broadcast_to([P, 2, 2, gi]),
                        Pi[:, si]
                        .unsqueeze(1)
                        .broadcast_to([P, 2, 2, gi]),
                        op=MULT,
                    )
                V.tensor_add(W2b[:], pw[:, 0], pw[:, 1])
                for si in range(2):
                    V.tensor_tensor(
                        pw[:, si],
                        W2b[:, :, si]
                        .unsqueeze(2)
                        .broadcast_to([P, 2, 2, gi]),
                        Bblk[:, :, si]
                        .unsqueeze(1)
                        .broadcast_to([P, 2, 2, gi]),
                        op=MULT,
                    )
                V.tensor_add(t4[:], pw[:, 0], pw[:, 1])
                V.tensor_sub(
                    Sc[:],
                    Sblk[:, 2:4, 2:4],
                    t4[:],
                )
                Scq = Sc[:].rearrange("p q r g -> p (q r) g")
                V.tensor_tensor(u2[:], Scq[:, 0:2], Scq[:, 3:1:-1], op=MULT)
                V.tensor_sub(d0[:], u2[:, 0], u2[:, 1])
                V.reciprocal_approx_fast(out=W4b[:, 0], in_=d0[:])
                V.tensor_scalar_mul(
                    W4b[:, 1:3], W4b[:, 0].unsqueeze(1).broadcast_to([P, 2, gi]), -1.0
                )
                V.tensor_copy(W4b[:, 3], W4b[:, 0])
                ib["_cont"] = (X2, W4b, Pi, pw, W2b, Sc, X22, X21n, t4)

            def emit_inv_b(b):
                ib = inv_st[b]
                (X2, W4b, Pi, pw, W2b, Sc, X22, X21n, t4) = ib["_cont"]
                V.tensor_tensor(
                    X22[:],
                    Sc[:, 1::-1, 1::-1],
                    W4b[:].rearrange("p (q r) g -> p q r g", q=2),
                    op=MULT,
                )
                for si in range(2):
                    V.tensor_tensor(
                        pw[:, si],
                        X22[:, :, si]
                        .unsqueeze(2)
                        .broadcast_to([P, 2, 2, gi]),
                        W2b[:, si]
                        .unsqueeze(1)
                        .broadcast_to([P, 2, 2, gi]),
                        op=MULT,
                    )
                V.tensor_add(X21n[:], pw[:, 0], pw[:, 1])
                Xblk = X2[:].rearrange("p (n m) g -> p n m g", n=4)
                V.tensor_scalar_mul(Xblk[:, 2:4, 0:2], X21n[:], -1.0)
                V.tensor_scalar_mul(
                    Xblk[:, 0:2, 2:4].rearrange("p n mp g -> p mp n g"),
                    X21n[:],
                    -1.0,
                )
                V.tensor_copy(Xblk[:, 2:4, 2:4], X22[:])
                for si in range(2):
                    V.tensor_tensor(
                        pw[:, si],
                        W2b[:, si]
                        .unsqueeze(2)
                        .broadcast_to([P, 2, 2, gi]),
                        X21n[:, si]
                        .unsqueeze(1)
                        .broadcast_to([P, 2, 2, gi]),
                        op=MULT,
                    )
                V.tensor_add(t4[:], pw[:, 0], pw[:, 1])
                V.tensor_add(
                    Xblk[:, 0:2, 0:2],
                    Pi[:],
                    t4[:],
                )

            def emit_KG(c):
                s = st[c]
                b = c // ipair
                ph = c % ipair
                X2 = inv_st[b]["X2"]
                Xh = X2[:, :, ph * g : (ph + 1) * g]
                prodK = prodp.tile([P, 8, 4, 4, g], fp16, tag="prodK", name="prodK")
                ENG("KG").tensor_tensor(
                    prodK[:].rearrange("p i n m g -> p (i n) m g"),
                    s["P12p"][:]
                    .rearrange("p i n g -> p (i n) g")
                    .unsqueeze(2)
                    .broadcast_to([P, 32, 4, g]),
                    Xh.unsqueeze(1).broadcast_to([P, 8, 16, g]),
                    op=MULT,
                )
                psK = psmain.tile([P, 8 * 4 * g], fp32, tag="ps", name="psK")
                for n in range(4):
                    nc.tensor.matmul(
                        psK[:],
                        id16,
                        prodK[:, :, n].rearrange("p i m g -> p i (m g)"),
                        start=(n == 0),
                        stop=(n == 3),
                    )
                KGo = outp.tile([P, g, 8, 4], fp32, tag="KGo", name="KGo")
                nc.scalar.copy(
                    KGo[:].rearrange("p g i m -> p (i m) g"),
                    psK[:].rearrange("p (im g) -> p im g", g=g),
                )
                nc.sync.dma_start(out=KGv[c], in_=KGo[:])

            def _qv_idx(t, si):
                return t[:, si]

            # waves: L(t) | T(t-1) | A(t-2) | P12(t-3) | S(t-4) |
            #        KG(t-4-ipair) | C(t-2) | inv(group ending at t-4)
            kskew = 5 + ipair
            for t in range(nchunk + kskew + 1):
                if t < nchunk:
                    emit_load(t)
                if 0 <= t - 1 < nchunk:
                    emit_transpose(t - 1)
                if 0 <= t - 2 < nchunk:
                    emit_A(t - 2)
                if 0 <= t - 3 < nchunk:
                    emit_P12(t - 3)
                if 0 <= t - 4 < nchunk:
                    emit_S(t - 4)
                if 0 <= t - 2 < nchunk:
                    emit_C(t - 2)
                last_b = nchunk // ipair - 1
                if 0 <= t - 4 < nchunk and (t - 4) % ipair == ipair - 1:
                    bb = (t - 4) // ipair
                    emit_inv(bb)
                    if bb == last_b:
                        # drain: nothing to smooth, finish the inverse now
                        emit_inv_b(bb)
                if 0 <= t - 5 < nchunk and (t - 5) % ipair == ipair - 1:
                    if (t - 5) // ipair != last_b:
                        emit_inv_b((t - 5) // ipair)
                # two KGs per wave starting one wave after inv_b
                o = t - kskew
                if o >= 0 and (o + 1) // ipair >= nchunk // ipair - 1:
                    o = o + 1  # last group's inverse finishes a wave earlier
                if o >= 0:
                    b2 = o // ipair
                    oo = o - b2 * ipair
                    if 0 <= oo < ipair // 2:
                        for c2 in (
                            b2 * ipair + 2 * oo,
                            b2 * ipair + 2 * oo + 1,
                        ):
                            if 0 <= c2 < nchunk:
                                emit_KG(c2)
                if 0 <= t - 1 < nchunk:
                    emit_transpose_tail(t - 1)

    nc.compile()
    return nc


def _get_nc():
    if "nc" not in _NC_CACHE:
        _NC_CACHE["nc"] = _build_nc()
    return _NC_CACHE["nc"]


def kernel(F, H, Sigma_previous, Q, R):
    from concourse.bass_utils import run_bass_kernel_spmd

    nc = _get_nc()
    in_maps = []
    for ci in range(NCORES):
        sl = slice(ci * B_CORE, (ci + 1) * B_CORE)
        in_maps.append(
            {
                "F": np.ascontiguousarray(F[sl], dtype=np.float32),
                "H": np.ascontiguousarray(H[sl], dtype=np.float32),
                "Sigma_previous": np.ascontiguousarray(
                    Sigma_previous[sl], dtype=np.float32
                ),
                "Q": np.ascontiguousarray(Q[sl], dtype=np.float32),
                "R": np.ascontiguousarray(R[sl], dtype=np.float32),
            }
        )
    res = run_bass_kernel_spmd(nc, in_maps, core_ids=list(range(NCORES)))
    return np.concatenate([r["KG"] for r in res.results], axis=0)
